# revision 31
# baseline (speedup 1.0000x reference)
# Trainium2 Bass kernel for Mixtral block-sparse MLP with HQQ 4-bit (int32-stored)
# group-quantized weights.
#
#   gate = silu(x @ dequant(w1).T); up = x @ dequant(w3).T
#   out  = (gate * up) @ dequant(w2).T
#
# Sharding: tensor-parallel over 8 cores on the intermediate dim I=14336
# (1792 rows of w1/w3 + 1792 cols of w2 per core).  Each core computes a
# full-shape [T, H] partial of the down-projection; the host sums the 8
# partials (cheap in numpy) instead of an on-device AllReduce.
#
# All matmuls run in fp8e4 DoubleRow mode (2 MACs/cell/cycle = 2x fp16):
#   -

# w1/w3/w2 are dequantized on the host, re-quantized per-output-column
#     to fp8e4 (scale 192/colmax), and streamed to the device as fp8.  The
#     column scale moves OUTSIDE the matmul (applied to the f32 PSUM output
#     during the silu evacuation via a host-pre-broadcast [128, col] tile).
#     No device-side dequant or dtype conversion at all.
#   - x is quantized to fp8e4 at scale 32 (x8) PLUS an fp8 residual e8 =
#     fp8(32x - x8) at the same scale.  The gate matmul uses x8 only; the
#     up matmul accumulates x8@W3 and e8@W3 into the same PSUM region,
#     recovering ~fp16 activation precision where it matters (up-path
#     errors pass through silu(gate) unattenuated; gate-path errors are
#     damped by silu').  Measured end-to-end rel err ~1.5e-2 vs the 2e-2
#     gate.
#   - phase 2 is unchanged: act -> fp8 (x 1/16), w2 fp8 (x16), DoubleRow.
# Device pipeline per core:
#   DMA w13 fp8 batch -> PE DR matmul1 (x8 full width + e8 up half, psum)
#   -> DVE col-scale + silu*up (fp8-range) -> PE transpose -> actT (fp8)
#   -> PE matmul2 (DoubleRow fp8) -> ACT evac -> DMA out (fp16).
# The first matmul is gated only on a 1-k-pair weight batch + small x8/e8
# blocks; weight DMAs stream chunk-contiguous (1-4KB per-partition packets);
# all 8 w2 chunks prefetch into the DMA gap between the end of w13
# streaming and phase 2.

import sys
from contextlib import ExitStack

import numpy as np

sys.path.insert(0, "/opt/trn_rl_repo")

import concourse.bacc as bacc
import concourse.mybir as mybir
import concourse.tile as tile

P = 128
GS = 64  # HQQ quant group size (along each weight's input dim)
F32 = mybir.dt.float32
AF = mybir.ActivationFunctionType
ALU = mybir.AluOpType
FP16 = mybir.dt.float16
FP8 = mybir.dt.float8e4
DR = mybir.MatmulPerfMode.DoubleRow

T, H, I, NCORES = 256, 4096, 14336, 8
IS = I // NCORES          # 1792 intermediate rows/cols per core
KT = H // P               # 32 k-tiles for matmul1
KP = KT // 2              # 16 DoubleRow k-pairs for matmul1
IT = IS // P              # 14 i-tiles for matmul2
TT = T // P               # 2 token tiles
CW = 512                  # matmul free-dim chunk width (1 psum bank of f32)
IC = CW // 2              # 256 gate + 256 up columns per chunk
NC1 = 2 * IS // CW        # 7 chunks of interleaved [w1|w3] columns
NH2 = H // CW             # 8 output column chunks


def build_mlp_kernel(tc, outs, ins, cfg):
    nc = tc.nc
    w13 = ins["w13"]        # [P, NC1, KP, 2, CW] fp8 (col-requantized, interleaved)
    cs = ins["cs"]          # [P, NC1, CW] fp16 col scales (pre-broadcast)
    w2 = ins["w2"]          # [P, NH2, IT//2, 2, CW] fp8 (DR-paired)
    x8T = ins["x8T"]        # [P, KP, 2, T] fp8
    e8T = ins["e8T"]        # [P, KP, 2, T] fp8 residual
    out_ext = outs["out"]   # [T, H] fp16 partial (host sums cores)

    ctx = ExitStack()
    with ctx:
        const = ctx.enter_context(tc.tile_pool(name="const", bufs=1))
        pst = ctx.enter_context(tc.tile_pool(name="pst", bufs=2, space="PSUM"))
        pout = ctx.enter_context(tc.tile_pool(name="pout", bufs=3, space="PSUM"))
        wtp = ctx.enter_context(tc.tile_pool(name="wt", bufs=int(cfg.get("WBUFS", 12))))
        w2p = ctx.enter_context(tc.tile_pool(name="w2p", bufs=NH2))
        miscp = ctx.enter_context(tc.tile_pool(name="misc", bufs=4))
        obp = ctx.enter_context(tc.tile_pool(name="ob", bufs=8))

        KBLK = int(cfg.get("KBLK", 4))   # k-pairs per weight DMA batch
        XBLK = 4                         # k-pairs per x8/e8 block
        NXB = KP // XBLK

        # x8/e8 block 0 gate the first matmuls; stream them on the scalar
        # queue so they run concurrently with the first weight batch (sync).
        x8b = [
            const.tile([P, XBLK, 2, T], FP8, tag=f"x8{b}", name="x8b")
            for b in range(NXB)
        ]
        e8b = [
            const.tile([P, XBLK, 2, T], FP8, tag=f"e8{b}", name="e8b")
            for b in range(NXB)
        ]
        nc.scalar.dma_start(x8b[0], x8T[:, 0:XBLK])
        ident = const.tile([P, P], FP16, name="ident")

        # PE warmup: the PE p-state ramps with activity (cold matmuls run
        # ~1.7x slower) and the first ~6us are DMA-ramp dead time anyway.
        # Spin the PE on a zeroed tile so the real matmuls start warm.
        warm = const.tile([P, 2, CW], FP8, name="warm")
        nc.gpsimd.memset(warm, 0.0)
        pwarm = pout.tile([P, TT, CW], F32, tag="po", name="pwarm")
        for _ in range(int(cfg.get("NWARM", 8))):
            nc.tensor.matmul(
                pwarm[:, 0], lhsT=warm[:, :, 0:P], rhs=warm, start=True,
                stop=True, perf_mode=DR,
            )

        cs_sb = const.tile([P, NC1, CW], FP16, name="cs_sb")
        # later x/e blocks + cs stream on the scalar queue behind block 0,
        # ordered by first-use time (x blocks feed the x8 pass of chunk 0,
        # e blocks the deferred e8 pass, cs/ident the act chains).
        _pend_dma = [
            lambda: nc.scalar.dma_start(x8b[1], x8T[:, XBLK:2 * XBLK]),
            lambda: nc.scalar.dma_start(e8b[0], e8T[:, 0:XBLK]),
            lambda: nc.scalar.dma_start(x8b[2], x8T[:, 2 * XBLK:3 * XBLK]),
            lambda: nc.scalar.dma_start(e8b[1], e8T[:, XBLK:2 * XBLK]),
            lambda: nc.scalar.dma_start(x8b[3], x8T[:, 3 * XBLK:4 * XBLK]),
            lambda: nc.scalar.dma_start(e8b[2], e8T[:, 2 * XBLK:3 * XBLK]),
            lambda: nc.scalar.dma_start(e8b[3], e8T[:, 3 * XBLK:4 * XBLK]),
            lambda: nc.scalar.dma_start(ident, ins["ident"]),
            lambda: nc.scalar.dma_start(cs_sb, cs),
        ]

        actT_sb = const.tile([P, IT, T], FP8, name="actT_sb")

        pend = []  # deferred tail work (SW pipeline: keeps PE stream dense)
        w2_gates = {}

        # ================= phase 1: gate/up + silu*up ====================
        # column layout: chunks [G0,U0,G1,U1,G2,U2,M] — 3 pairs of full
        # 512-wide gate/up chunks plus one mixed [256g|256u] chunk.  The e8
        # residual pass then runs at N=512 where its LDWEIGHTS is fully
        # hidden under the matmul stream (at N=256 it exposes ~28ns/op).
        def issue_chunk_dma(c, first=False):
            # chunk 0's tail batches ride the gpsimd queue: its cold-start
            # ramp finishes right around their need-time, and the sync
            # queue reaches chunk 1 two slots earlier.
            kb = [2, 2, 4, 4, 4] if first else [KBLK] * (KP // KBLK)
            qs = [nc.sync, nc.sync, nc.sync, nc.gpsimd, nc.gpsimd] if first \
                else [nc.sync] * len(kb)
            wbs = []
            k0 = 0
            for nk, q in zip(kb, qs):
                wb = wtp.tile([P, KBLK, 2, CW], FP8, tag="wt", name="wb")
                q.dma_start(wb[:, :nk], w13[:, c, k0:k0 + nk])
                if _pend_dma:
                    _pend_dma.pop(0)()
                if not wbs and c in (5, 6):
                    w2_gates[c] = wb
                wbs.append((k0, wb))
                k0 += nk
            if first:
                while _pend_dma:
                    _pend_dma.pop(0)()
            return wbs

        def _lookup(wbs, kp):
            for k0, wb in reversed(wbs):
                if kp >= k0:
                    return wb, kp - k0

        def new_pot(name):
            pot = pout.tile([P, TT, CW], F32, tag="po", name=name)
            return [pot[:, t] for t in range(TT)]

        def run_g(po, wbs):
            # gate chunk: x8 pass only
            for kp in range(KP):
                wb, ki = _lookup(wbs, kp)
                xb, j = x8b[kp // XBLK], kp % XBLK
                for t in range(TT):
                    nc.tensor.matmul(
                        po[t],
                        lhsT=xb[:, j, :, t * P:(t + 1) * P],
                        rhs=wb[:, ki],
                        start=(kp == 0),
                        stop=(kp == KP - 1),
                        perf_mode=DR,
                    )

        def run_u(po, wbs):
            # up chunk: x8 pass + full-width e8 residual pass
            for kp in range(KP):
                wb, ki = _lookup(wbs, kp)
                xb, eb, j = x8b[kp // XBLK], e8b[kp // XBLK], kp % XBLK
                for t in range(TT):
                    nc.tensor.matmul(
                        po[t],
                        lhsT=xb[:, j, :, t * P:(t + 1) * P],
                        rhs=wb[:, ki],
                        start=(kp == 0),
                        stop=False,
                        perf_mode=DR,
                    )
                for t in range(TT):
                    nc.tensor.matmul(
                        po[t],
                        lhsT=eb[:, j, :, t * P:(t + 1) * P],
                        rhs=wb[:, ki],
                        start=False,
                        stop=(kp == KP - 1),
                        perf_mode=DR,
                    )

        def run_m(po, wbs):
            # mixed chunk: x8 full width, e8 on the up half only
            for kp in range(KP):
                wb, ki = _lookup(wbs, kp)
                xb, eb, j = x8b[kp // XBLK], e8b[kp // XBLK], kp % XBLK
                for t in range(TT):
                    nc.tensor.matmul(
                        po[t],
                        lhsT=xb[:, j, :, t * P:(t + 1) * P],
                        rhs=wb[:, ki],
                        start=(kp == 0),
                        stop=False,
                        perf_mode=DR,
                    )
                for t in range(TT):
                    nc.tensor.matmul(
                        po[t][:, IC:],
                        lhsT=eb[:, j, :, t * P:(t + 1) * P],
                        rhs=wb[:, ki, :, IC:],
                        start=False,
                        stop=(kp == KP - 1),
                        perf_mode=DR,
                    )

        def make_act_pair(i, pg, pu):
            def act_pair():
                for t in range(TT):
                    g = miscp.tile([P, CW], F32, tag="g", name="g")
                    nc.vector.tensor_tensor(g, pg[t], cs_sb[:, 2 * i], ALU.mult)
                    u = miscp.tile([P, CW], F32, tag="u", name="u")
                    nc.vector.tensor_tensor(u, pu[t], cs_sb[:, 2 * i + 1], ALU.mult)
                    sig = miscp.tile([P, CW], F32, tag="sig", name="sig")
                    nc.scalar.activation(sig, g, AF.Sigmoid)
                    silu = miscp.tile([P, CW], F32, tag="silu", name="silu")
                    nc.vector.tensor_tensor(silu, g, sig, ALU.mult)
                    acth = miscp.tile([P, CW], FP16, tag="acth", name="acth")
                    nc.vector.scalar_tensor_tensor(
                        acth, u, 1.0 / 16.0, silu, ALU.mult, ALU.mult
                    )
                    ps = pst.tile([P, CW], FP16, tag="pst", name="ps")
                    for h in range(4):
                        nc.tensor.transpose(
                            ps[:, h * P:(h + 1) * P],
                            acth[:, h * P:(h + 1) * P],
                            ident,
                        )
                    nc.vector.tensor_copy(
                        out=actT_sb[:, 4 * i:4 * i + 4, t * P:(t + 1) * P],
                        in_=ps.rearrange("p (a b) -> p a b", b=P),
                    )
            return act_pair

        def make_act_m(po):
            def act_m():
                for t in range(TT):
                    gup = miscp.tile([P, CW], F32, tag="g", name="gup")
                    nc.vector.tensor_tensor(gup, po[t], cs_sb[:, 6], ALU.mult)
                    sig = miscp.tile([P, IC], F32, tag="sig", name="sig")
                    nc.scalar.activation(sig, gup[:, :IC], AF.Sigmoid)
                    silu = miscp.tile([P, IC], F32, tag="silu", name="silu")
                    nc.vector.tensor_tensor(silu, gup[:, :IC], sig, ALU.mult)
                    acth = miscp.tile([P, IC], FP16, tag="acth", name="acth")
                    nc.vector.scalar_tensor_tensor(
                        acth, gup[:, IC:], 1.0 / 16.0, silu, ALU.mult, ALU.mult
                    )
                    ps = pst.tile([P, IC], FP16, tag="pst", name="psm")
                    for h in range(2):
                        nc.tensor.transpose(
                            ps[:, h * P:(h + 1) * P],
                            acth[:, h * P:(h + 1) * P],
                            ident,
                        )
                    nc.vector.tensor_copy(
                        out=actT_sb[:, 12:14, t * P:(t + 1) * P],
                        in_=ps.rearrange("p (a b) -> p a b", b=P),
                    )
            return act_m

        # PE order: G0, U0, G1, act0, U1, G2, act1, U2, M, act2, actM —
        # each act issues only after the next chunk's matmuls so its PE
        # transposes never head-of-line block the matmul stream, while the
        # 3-deep PSUM ring always has a freed slot by allocation time.
        wbs = issue_chunk_dma(0, first=True)
        pg = new_pot("pg")
        run_g(pg, wbs)
        for i in range(3):
            pu = new_pot("pu")
            run_u(pu, issue_chunk_dma(2 * i + 1))
            pend.append(make_act_pair(i, pg, pu))
            if i < 2:
                pg = new_pot("pg")
                run_g(pg, issue_chunk_dma(2 * i + 2))
                pend.pop(0)()
        pm = new_pot("pm")
        run_m(pm, issue_chunk_dma(6))
        while pend:
            pend.pop(0)()
        act_m = make_act_m(pm)

        # ================= phase 2: down-projection partial ==============
        NP2 = IT // 2  # 7 DoubleRow k-pairs

        # w2 chunks stream on the (otherwise idle, in-order) gpsimd queue,
        # gated by a tiny WAW write into the destination tile that reads a
        # late-phase-1 w13 tile.  Without the gate the Tile scheduler hoists
        # the dep-free DMAs to t=0 where they starve the phase-1 weight
        # stream; gating on a busy engine instead delays them until ~90us.
        # hc 0-3 unlock with chunk 5's first batch, hc 4-7 with chunk 6's.
        wbs2 = []
        for hc in range(NH2):
            wb2 = w2p.tile([P, NP2, 2, CW], FP8, tag="w2", name="wb2")
            gate = w2_gates[5 if hc < 4 else 6]
            nc.gpsimd.tensor_copy(out=wb2[:, 0, 0, 0:16], in_=gate[:, 0, 0, 0:16])
            nc.gpsimd.dma_start(wb2, w2[:, hc])
            wbs2.append(wb2)

        for hc in range(NH2):
            pot2 = pout.tile([P, TT, CW], F32, tag="po", name="pot2")
            po2 = [pot2[:, t] for t in range(TT)]
            wb = wbs2[hc]
            for b in range(NP2):
                if hc == 0 and b == NP2 - 1:
                    # issue actM only now: its PE transposes sit behind six
                    # hc0 matmuls, so the DVE act chain for the mixed chunk
                    # runs concurrently instead of head-of-line blocking.
                    act_m()
                for t in range(TT):
                    nc.tensor.matmul(
                        po2[t],
                        lhsT=actT_sb[:, 2 * b:2 * b + 2, t * P:(t + 1) * P],
                        rhs=wb[:, b],
                        start=(b == 0),
                        stop=(b == NP2 - 1),
                        perf_mode=DR,
                    )

            def tail2(po2=po2, hc=hc):
                for t in range(TT):
                    ob = obp.tile([P, CW], FP16, tag="ob", name="ob")
                    nc.any.tensor_copy(out=ob, in_=po2[t])
                    nc.sync.dma_start(
                        out_ext[t * P:(t + 1) * P, hc * CW:(hc + 1) * CW], ob
                    )

            pend.append(tail2)
            if len(pend) >= 2:
                pend.pop(0)()
        while pend:
            pend.pop(0)()


# ---------------------------------------------------------------------------
# host side
# ---------------------------------------------------------------------------

FULL_CFG = dict(WBUFS=16, KBLK=4)


def build_nc(cfg):
    nc = bacc.Bacc(
        "TRN2",
        target_bir_lowering=False,
        debug=False,
        enable_asserts=False,
        num_devices=NCORES,
    )
    ins = {
        "x8T": nc.dram_tensor("x8T", [P, KP, 2, T], FP8, kind="ExternalInput").ap(),
        "e8T": nc.dram_tensor("e8T", [P, KP, 2, T], FP8, kind="ExternalInput").ap(),
        "ident": nc.dram_tensor("ident", [P, P], FP16, kind="ExternalInput").ap(),
        "w13": nc.dram_tensor(
            "w13", [P, NC1, KP, 2, CW], FP8, kind="ExternalInput"
        ).ap(),
        "cs": nc.dram_tensor("cs", [P, NC1, CW], FP16, kind="ExternalInput").ap(),
        "w2": nc.dram_tensor(
            "w2", [P, NH2, IT // 2, 2, CW], FP8, kind="ExternalInput"
        ).ap(),
    }
    outs = {"out": nc.dram_tensor("out", [T, H], FP16, kind="ExternalOutput").ap()}
    with tile.TileContext(nc) as tc:
        build_mlp_kernel(tc, outs, ins, cfg)
    nc.compile()
    return nc


def _dequant(wq, scale, zero):
    out_dim, in_dim = wq.shape
    g = in_dim // GS
    w = (wq.astype(np.float32).reshape(out_dim, g, GS) - zero[:, :, None]) \
        * scale[:, :, None]
    return w.reshape(out_dim, in_dim)


def _stripe(a, nt):
    # [(k p), n] -> [p, k, n] so each SBUF partition's data is contiguous
    return np.ascontiguousarray(
        a.reshape(nt, P, a.shape[1]).transpose(1, 0, 2)
    )


def make_in_maps(inputs):
    import ml_dtypes

    E4 = ml_dtypes.float8_e4m3fn
    x = np.asarray(inputs["x"], dtype=np.float32)
    x32 = 32.0 * x
    x8v = np.clip(x32, -240, 240).astype(E4)
    e8v = np.clip(x32 - x8v.astype(np.float32), -240, 240).astype(E4)
    x8T = np.ascontiguousarray(
        _stripe(np.ascontiguousarray(x8v.T), KT).reshape(P, KP, 2, T)
    )
    e8T = np.ascontiguousarray(
        _stripe(np.ascontiguousarray(e8v.T), KT).reshape(P, KP, 2, T)
    )
    ident_np = np.eye(P, dtype=np.float16)

    w1 = _dequant(inputs["w1_q"], inputs["w1_scale"], inputs["w1_zero"])
    w3 = _dequant(inputs["w3_q"], inputs["w3_scale"], inputs["w3_zero"])
    w2 = _dequant(inputs["w2_q"], inputs["w2_scale"], inputs["w2_zero"])

    in_maps = []
    for c in range(NCORES):
        sl = slice(c * IS, (c + 1) * IS)
        w1T = w1[sl].T   # [H, IS] f32
        w3T = w3[sl].T   # [H, IS] f32
        # chunk layout [G0,U0,G1,U1,G2,U2,M]: 3 pairs of full CW-wide
        # gate/up chunks, then one mixed [256g|256u] chunk (act column
        # order is unchanged: pair i covers original act cols i*CW..)
        w13T = np.empty((H, 2 * IS), dtype=np.float32)
        NF = IS // CW  # 3 full pairs
        for i in range(NF):
            w13T[:, (2 * i) * CW:(2 * i + 1) * CW] = w1T[:, i * CW:(i + 1) * CW]
            w13T[:, (2 * i + 1) * CW:(2 * i + 2) * CW] = w3T[:, i * CW:(i + 1) * CW]
        w13T[:, 2 * NF * CW:2 * NF * CW + IC] = w1T[:, NF * CW:]
        w13T[:, 2 * NF * CW + IC:] = w3T[:, NF * CW:]
        # per-column fp8 re-quantization; scale applied to the psum output
        colmax = np.abs(w13T).max(axis=0)
        colmax[colmax == 0] = 1.0
        w13q = np.clip(w13T * (192.0 / colmax), -240, 240).astype(E4)
        csB = np.ascontiguousarray(
            np.broadcast_to(
                (colmax / (192.0 * 32.0)).astype(np.float16).reshape(NC1, CW),
                (P, NC1, CW),
            )
        )
        # [p, k, c*CW] -> [p, c, kp, 2, CW] chunk-contiguous per partition
        w13_s = np.ascontiguousarray(
            _stripe(w13q, KT)
            .reshape(P, KT, NC1, CW)
            .transpose(0, 2, 1, 3)
            .reshape(P, NC1, KP, 2, CW)
        )
        w2T = np.ascontiguousarray(w2[:, sl].T) * 16.0  # [IS, H] f32, fp8-range
        # act/16 * w2*16 -> unit output scale: evac is a plain copy
        w2_s = np.ascontiguousarray(
            _stripe(np.clip(w2T, -240, 240), IT)
            .reshape(P, IT, NH2, CW)
            .transpose(0, 2, 1, 3)                  # [P, hc, ik, CW]
            .reshape(P, NH2, IT // 2, 2, CW)        # ik -> (pair, j)
        ).astype(E4)
        in_maps.append(
            {
                "x8T": x8T,
                "e8T": e8T,
                "ident": ident_np,
                "w13": w13_s,
                "cs": csB,
                "w2": w2_s,
            }
        )
    return in_maps


_CACHE = {}


def run_on_hw(inputs, cfg=None, trace=False, trace_kwargs=None):
    from concourse.bass_utils import run_bass_kernel_spmd

    cfg = dict(FULL_CFG if cfg is None else cfg)
    key = tuple(sorted(cfg.items()))
    if key not in _CACHE:
        _CACHE[key] = build_nc(cfg)
    nc = _CACHE[key]
    in_maps = make_in_maps(inputs)
    res = run_bass_kernel_spmd(
        nc,
        in_maps,
        list(range(NCORES)),
        trace=trace,
        **(trace_kwargs or {}),
    )
    return res


def gather_out(res):
    return np.sum(
        [np.asarray(res.results[c]["out"], dtype=np.float32) for c in range(NCORES)],
        axis=0,
    )


def kernel(**inputs) -> np.ndarray:
    res = run_on_hw(inputs)
    return gather_out(res)


# revision 32
# speedup vs baseline: 1.0329x; 1.0329x over previous
# Trainium2 Bass kernel for Mixtral block-sparse MLP with HQQ 4-bit (int32-stored)
# group-quantized weights.
#
#   gate = silu(x @ dequant(w1).T); up = x @ dequant(w3).T
#   out  = (gate * up) @ dequant(w2).T
#
# Sharding: tensor-parallel over 8 cores on the intermediate dim I=14336
# (1792 rows of w1/w3 + 1792 cols of w2 per core).  Each core computes a
# full-shape [T, H] partial of the down-projection; the host sums the 8
# partials (cheap in numpy) instead of an on-device AllReduce.
#
# All matmuls run in fp8e4 DoubleRow mode (2 MACs/cell/cycle = 2x fp16):
#   -

# w1/w3/w2 are dequantized on the host, re-quantized per-output-column
#     to fp8e4 (scale 192/colmax), and streamed to the device as fp8.  The
#     column scale moves OUTSIDE the matmul (applied to the f32 PSUM output
#     during the silu evacuation via a host-pre-broadcast [128, col] tile).
#     No device-side dequant or dtype conversion at all.
#   - x is quantized to fp8e4 at scale 32 (x8) PLUS an fp8 residual e8 =
#     fp8(32x - x8) at the same scale.  The gate matmul uses x8 only; the
#     up matmul accumulates x8@W3 and e8@W3 into the same PSUM region,
#     recovering ~fp16 activation precision where it matters (up-path
#     errors pass through silu(gate) unattenuated; gate-path errors are
#     damped by silu').  Measured end-to-end rel err ~1.5e-2 vs the 2e-2
#     gate.
#   - phase 2 is unchanged: act -> fp8 (x 1/16), w2 fp8 (x16), DoubleRow.
# Device pipeline per core:
#   DMA w13 fp8 batch -> PE DR matmul1 (x8 full width + e8 up half, psum)
#   -> DVE col-scale + silu*up (fp8-range) -> PE transpose -> actT (fp8)
#   -> PE matmul2 (DoubleRow fp8) -> ACT evac -> DMA out (fp16).
# The first matmul is gated only on a 1-k-pair weight batch + small x8/e8
# blocks; weight DMAs stream chunk-contiguous (1-4KB per-partition packets);
# all 8 w2 chunks prefetch into the DMA gap between the end of w13
# streaming and phase 2.

import sys
from contextlib import ExitStack

import numpy as np

sys.path.insert(0, "/opt/trn_rl_repo")

import concourse.bacc as bacc
import concourse.mybir as mybir
import concourse.tile as tile

P = 128
GS = 64  # HQQ quant group size (along each weight's input dim)
F32 = mybir.dt.float32
AF = mybir.ActivationFunctionType
ALU = mybir.AluOpType
FP16 = mybir.dt.float16
FP8 = mybir.dt.float8e4
DR = mybir.MatmulPerfMode.DoubleRow

T, H, I, NCORES = 256, 4096, 14336, 8
IS = I // NCORES          # 1792 intermediate rows/cols per core
KT = H // P               # 32 k-tiles for matmul1
KP = KT // 2              # 16 DoubleRow k-pairs for matmul1
IT = IS // P              # 14 i-tiles for matmul2
TT = T // P               # 2 token tiles
CW = 512                  # matmul free-dim chunk width (1 psum bank of f32)
IC = CW // 2              # 256 gate + 256 up columns per chunk
NC1 = 2 * IS // CW        # 7 chunks of interleaved [w1|w3] columns
NH2 = H // CW             # 8 output column chunks


def build_mlp_kernel(tc, outs, ins, cfg):
    nc = tc.nc
    w13 = ins["w13"]        # [P, NC1, KP, 2, CW] fp8 (col-requantized, interleaved)
    cs = ins["cs"]          # [P, NC1, CW] fp16 col scales (pre-broadcast)
    w2 = ins["w2"]          # [P, NH2, IT//2, 2, CW] fp8 (DR-paired)
    x8T = ins["x8T"]        # [P, KP, 2, T] fp8
    e8T = ins["e8T"]        # [P, KP, 2, T] fp8 residual
    out_ext = outs["out"]   # [T, H] fp16 partial (host sums cores)

    ctx = ExitStack()
    with ctx:
        const = ctx.enter_context(tc.tile_pool(name="const", bufs=1))
        pst = ctx.enter_context(tc.tile_pool(name="pst", bufs=2, space="PSUM"))
        pout = ctx.enter_context(tc.tile_pool(name="pout", bufs=3, space="PSUM"))
        wtp = ctx.enter_context(tc.tile_pool(name="wt", bufs=int(cfg.get("WBUFS", 12))))
        w2p = ctx.enter_context(tc.tile_pool(name="w2p", bufs=NH2))
        miscp = ctx.enter_context(tc.tile_pool(name="misc", bufs=4))
        obp = ctx.enter_context(tc.tile_pool(name="ob", bufs=8))

        KBLK = int(cfg.get("KBLK", 4))   # k-pairs per weight DMA batch
        XBLK = 4                         # k-pairs per x8/e8 block
        NXB = KP // XBLK

        # x8/e8 block 0 gate the first matmuls; stream them on the scalar
        # queue so they run concurrently with the first weight batch (sync).
        x8b = [
            const.tile([P, XBLK, 2, T], FP8, tag=f"x8{b}", name="x8b")
            for b in range(NXB)
        ]
        e8b = [
            const.tile([P, XBLK, 2, T], FP8, tag=f"e8{b}", name="e8b")
            for b in range(NXB)
        ]
        nc.scalar.dma_start(x8b[0], x8T[:, 0:XBLK])
        ident = const.tile([P, P], FP16, name="ident")

        # PE warmup: the PE p-state ramps with activity (cold matmuls run
        # ~1.7x slower) and the first ~6us are DMA-ramp dead time anyway.
        # Spin the PE on a zeroed tile so the real matmuls start warm.
        warm = const.tile([P, 2, CW], FP8, name="warm")
        nc.gpsimd.memset(warm, 0.0)
        pwarm = pout.tile([P, TT, CW], F32, tag="po", name="pwarm")
        for _ in range(int(cfg.get("NWARM", 8))):
            nc.tensor.matmul(
                pwarm[:, 0], lhsT=warm[:, :, 0:P], rhs=warm, start=True,
                stop=True, perf_mode=DR,
            )

        cs_sb = const.tile([P, NC1, CW], FP16, name="cs_sb")
        # later x/e blocks + cs stream on the scalar queue behind block 0,
        # ordered by first-use time (x blocks feed the x8 pass of chunk 0,
        # e blocks the deferred e8 pass, cs/ident the act chains).
        _pend_dma = [
            lambda: nc.scalar.dma_start(x8b[1], x8T[:, XBLK:2 * XBLK]),
            lambda: nc.scalar.dma_start(e8b[0], e8T[:, 0:XBLK]),
            lambda: nc.scalar.dma_start(x8b[2], x8T[:, 2 * XBLK:3 * XBLK]),
            lambda: nc.scalar.dma_start(e8b[1], e8T[:, XBLK:2 * XBLK]),
            lambda: nc.scalar.dma_start(x8b[3], x8T[:, 3 * XBLK:4 * XBLK]),
            lambda: nc.scalar.dma_start(e8b[2], e8T[:, 2 * XBLK:3 * XBLK]),
            lambda: nc.scalar.dma_start(e8b[3], e8T[:, 3 * XBLK:4 * XBLK]),
            lambda: nc.scalar.dma_start(ident, ins["ident"]),
            lambda: nc.scalar.dma_start(cs_sb, cs),
        ]

        actT_sb = const.tile([P, IT, T], FP8, name="actT_sb")

        pend = []  # deferred tail work (SW pipeline: keeps PE stream dense)
        w2_gates = {}

        # ================= phase 1: gate/up + silu*up ====================
        # column layout: chunks [G0,U0,G1,U1,G2,U2,M] — 3 pairs of full
        # 512-wide gate/up chunks plus one mixed [256g|256u] chunk.  The e8
        # residual pass then runs at N=512 where its LDWEIGHTS is fully
        # hidden under the matmul stream (at N=256 it exposes ~28ns/op).
        def issue_chunk_dma(c, first=False):
            kb = [2, 2, 4, 4, 4] if first else [KBLK] * (KP // KBLK)
            qs = [nc.sync] * len(kb)
            wbs = []
            k0 = 0
            for nk, q in zip(kb, qs):
                wb = wtp.tile([P, KBLK, 2, CW], FP8, tag="wt", name="wb")
                q.dma_start(wb[:, :nk], w13[:, c, k0:k0 + nk])
                if _pend_dma:
                    _pend_dma.pop(0)()
                if not wbs and c in (5, 6):
                    w2_gates[c] = wb
                wbs.append((k0, wb))
                k0 += nk
            if first:
                while _pend_dma:
                    _pend_dma.pop(0)()
            return wbs

        def _lookup(wbs, kp):
            for k0, wb in reversed(wbs):
                if kp >= k0:
                    return wb, kp - k0

        def new_pot(name):
            pot = pout.tile([P, TT, CW], F32, tag="po", name=name)
            return [pot[:, t] for t in range(TT)]

        def run_g(po, wbs):
            # gate chunk: x8 pass only
            for kp in range(KP):
                wb, ki = _lookup(wbs, kp)
                xb, j = x8b[kp // XBLK], kp % XBLK
                for t in range(TT):
                    nc.tensor.matmul(
                        po[t],
                        lhsT=xb[:, j, :, t * P:(t + 1) * P],
                        rhs=wb[:, ki],
                        start=(kp == 0),
                        stop=(kp == KP - 1),
                        perf_mode=DR,
                    )

        def run_u(po, wbs):
            # up chunk: x8 pass + full-width e8 residual pass
            for kp in range(KP):
                wb, ki = _lookup(wbs, kp)
                xb, eb, j = x8b[kp // XBLK], e8b[kp // XBLK], kp % XBLK
                for t in range(TT):
                    nc.tensor.matmul(
                        po[t],
                        lhsT=xb[:, j, :, t * P:(t + 1) * P],
                        rhs=wb[:, ki],
                        start=(kp == 0),
                        stop=False,
                        perf_mode=DR,
                    )
                for t in range(TT):
                    nc.tensor.matmul(
                        po[t],
                        lhsT=eb[:, j, :, t * P:(t + 1) * P],
                        rhs=wb[:, ki],
                        start=False,
                        stop=(kp == KP - 1),
                        perf_mode=DR,
                    )

        def run_m(po, wbs):
            # mixed chunk: x8 full width, e8 on the up half only
            for kp in range(KP):
                wb, ki = _lookup(wbs, kp)
                xb, eb, j = x8b[kp // XBLK], e8b[kp // XBLK], kp % XBLK
                for t in range(TT):
                    nc.tensor.matmul(
                        po[t],
                        lhsT=xb[:, j, :, t * P:(t + 1) * P],
                        rhs=wb[:, ki],
                        start=(kp == 0),
                        stop=False,
                        perf_mode=DR,
                    )
                for t in range(TT):
                    nc.tensor.matmul(
                        po[t][:, IC:],
                        lhsT=eb[:, j, :, t * P:(t + 1) * P],
                        rhs=wb[:, ki, :, IC:],
                        start=False,
                        stop=(kp == KP - 1),
                        perf_mode=DR,
                    )

        def make_act_pair(i, pg, pu):
            def act_pair():
                for t in range(TT):
                    g = miscp.tile([P, CW], F32, tag="g", name="g")
                    nc.vector.tensor_tensor(g, pg[t], cs_sb[:, 2 * i], ALU.mult)
                    u = miscp.tile([P, CW], F32, tag="u", name="u")
                    nc.vector.tensor_tensor(u, pu[t], cs_sb[:, 2 * i + 1], ALU.mult)
                    sig = miscp.tile([P, CW], F32, tag="sig", name="sig")
                    nc.scalar.activation(sig, g, AF.Sigmoid)
                    silu = miscp.tile([P, CW], F32, tag="silu", name="silu")
                    nc.vector.tensor_tensor(silu, g, sig, ALU.mult)
                    acth = miscp.tile([P, CW], FP16, tag="acth", name="acth")
                    nc.vector.scalar_tensor_tensor(
                        acth, u, 1.0 / 16.0, silu, ALU.mult, ALU.mult
                    )
                    ps = pst.tile([P, CW], FP16, tag="pst", name="ps")
                    for h in range(4):
                        nc.tensor.transpose(
                            ps[:, h * P:(h + 1) * P],
                            acth[:, h * P:(h + 1) * P],
                            ident,
                        )
                    nc.vector.tensor_copy(
                        out=actT_sb[:, 4 * i:4 * i + 4, t * P:(t + 1) * P],
                        in_=ps.rearrange("p (a b) -> p a b", b=P),
                    )
            return act_pair

        def make_act_m(po):
            def act_m():
                for t in range(TT):
                    gup = miscp.tile([P, CW], F32, tag="g", name="gup")
                    nc.vector.tensor_tensor(gup, po[t], cs_sb[:, 6], ALU.mult)
                    sig = miscp.tile([P, IC], F32, tag="sig", name="sig")
                    nc.scalar.activation(sig, gup[:, :IC], AF.Sigmoid)
                    silu = miscp.tile([P, IC], F32, tag="silu", name="silu")
                    nc.vector.tensor_tensor(silu, gup[:, :IC], sig, ALU.mult)
                    acth = miscp.tile([P, IC], FP16, tag="acth", name="acth")
                    nc.vector.scalar_tensor_tensor(
                        acth, gup[:, IC:], 1.0 / 16.0, silu, ALU.mult, ALU.mult
                    )
                    ps = pst.tile([P, IC], FP16, tag="pst", name="psm")
                    for h in range(2):
                        nc.tensor.transpose(
                            ps[:, h * P:(h + 1) * P],
                            acth[:, h * P:(h + 1) * P],
                            ident,
                        )
                    nc.vector.tensor_copy(
                        out=actT_sb[:, 12:14, t * P:(t + 1) * P],
                        in_=ps.rearrange("p (a b) -> p a b", b=P),
                    )
            return act_m

        # PE order: G0, U0, G1, act0, U1, G2, act1, U2, M, act2, actM —
        # each act issues only after the next chunk's matmuls so its PE
        # transposes never head-of-line block the matmul stream, while the
        # 3-deep PSUM ring always has a freed slot by allocation time.
        wbs = issue_chunk_dma(0, first=True)
        pg = new_pot("pg")
        run_g(pg, wbs)
        for i in range(3):
            pu = new_pot("pu")
            run_u(pu, issue_chunk_dma(2 * i + 1))
            pend.append(make_act_pair(i, pg, pu))
            if i < 2:
                pg = new_pot("pg")
                run_g(pg, issue_chunk_dma(2 * i + 2))
                pend.pop(0)()
        pm = new_pot("pm")
        run_m(pm, issue_chunk_dma(6))
        while pend:
            pend.pop(0)()
        act_m = make_act_m(pm)

        # ================= phase 2: down-projection partial ==============
        NP2 = IT // 2  # 7 DoubleRow k-pairs

        # w2 chunks stream on the (otherwise idle, in-order) gpsimd queue,
        # gated by a tiny WAW write into the destination tile that reads a
        # late-phase-1 w13 tile.  Without the gate the Tile scheduler hoists
        # the dep-free DMAs to t=0 where they starve the phase-1 weight
        # stream; gating on a busy engine instead delays them until ~90us.
        # hc 0-3 unlock with chunk 5's first batch, hc 4-7 with chunk 6's.
        wbs2 = []
        for hc in range(NH2):
            wb2 = w2p.tile([P, NP2, 2, CW], FP8, tag="w2", name="wb2")
            gate = w2_gates[5 if hc < 4 else 6]
            nc.gpsimd.tensor_copy(out=wb2[:, 0, 0, 0:16], in_=gate[:, 0, 0, 0:16])
            nc.gpsimd.dma_start(wb2, w2[:, hc])
            wbs2.append(wb2)

        for hc in range(NH2):
            pot2 = pout.tile([P, TT, CW], F32, tag="po", name="pot2")
            po2 = [pot2[:, t] for t in range(TT)]
            wb = wbs2[hc]
            for b in range(NP2):
                if hc == 0 and b == NP2 - 1:
                    # issue actM only now: its PE transposes sit behind six
                    # hc0 matmuls, so the DVE act chain for the mixed chunk
                    # runs concurrently instead of head-of-line blocking.
                    act_m()
                for t in range(TT):
                    nc.tensor.matmul(
                        po2[t],
                        lhsT=actT_sb[:, 2 * b:2 * b + 2, t * P:(t + 1) * P],
                        rhs=wb[:, b],
                        start=(b == 0),
                        stop=(b == NP2 - 1),
                        perf_mode=DR,
                    )

            def tail2(po2=po2, hc=hc):
                for t in range(TT):
                    ob = obp.tile([P, CW], FP16, tag="ob", name="ob")
                    nc.any.tensor_copy(out=ob, in_=po2[t])
                    nc.sync.dma_start(
                        out_ext[t * P:(t + 1) * P, hc * CW:(hc + 1) * CW], ob
                    )

            pend.append(tail2)
            if len(pend) >= 2:
                pend.pop(0)()
        while pend:
            pend.pop(0)()


# ---------------------------------------------------------------------------
# host side
# ---------------------------------------------------------------------------

FULL_CFG = dict(WBUFS=16, KBLK=4)


def build_nc(cfg):
    nc = bacc.Bacc(
        "TRN2",
        target_bir_lowering=False,
        debug=False,
        enable_asserts=False,
        num_devices=NCORES,
    )
    ins = {
        "x8T": nc.dram_tensor("x8T", [P, KP, 2, T], FP8, kind="ExternalInput").ap(),
        "e8T": nc.dram_tensor("e8T", [P, KP, 2, T], FP8, kind="ExternalInput").ap(),
        "ident": nc.dram_tensor("ident", [P, P], FP16, kind="ExternalInput").ap(),
        "w13": nc.dram_tensor(
            "w13", [P, NC1, KP, 2, CW], FP8, kind="ExternalInput"
        ).ap(),
        "cs": nc.dram_tensor("cs", [P, NC1, CW], FP16, kind="ExternalInput").ap(),
        "w2": nc.dram_tensor(
            "w2", [P, NH2, IT // 2, 2, CW], FP8, kind="ExternalInput"
        ).ap(),
    }
    outs = {"out": nc.dram_tensor("out", [T, H], FP16, kind="ExternalOutput").ap()}
    with tile.TileContext(nc) as tc:
        build_mlp_kernel(tc, outs, ins, cfg)
    nc.compile()
    return nc


def _dequant(wq, scale, zero):
    out_dim, in_dim = wq.shape
    g = in_dim // GS
    w = (wq.astype(np.float32).reshape(out_dim, g, GS) - zero[:, :, None]) \
        * scale[:, :, None]
    return w.reshape(out_dim, in_dim)


def _stripe(a, nt):
    # [(k p), n] -> [p, k, n] so each SBUF partition's data is contiguous
    return np.ascontiguousarray(
        a.reshape(nt, P, a.shape[1]).transpose(1, 0, 2)
    )


def make_in_maps(inputs):
    import ml_dtypes

    E4 = ml_dtypes.float8_e4m3fn
    x = np.asarray(inputs["x"], dtype=np.float32)
    x32 = 32.0 * x
    x8v = np.clip(x32, -240, 240).astype(E4)
    e8v = np.clip(x32 - x8v.astype(np.float32), -240, 240).astype(E4)
    x8T = np.ascontiguousarray(
        _stripe(np.ascontiguousarray(x8v.T), KT).reshape(P, KP, 2, T)
    )
    e8T = np.ascontiguousarray(
        _stripe(np.ascontiguousarray(e8v.T), KT).reshape(P, KP, 2, T)
    )
    ident_np = np.eye(P, dtype=np.float16)

    w1 = _dequant(inputs["w1_q"], inputs["w1_scale"], inputs["w1_zero"])
    w3 = _dequant(inputs["w3_q"], inputs["w3_scale"], inputs["w3_zero"])
    w2 = _dequant(inputs["w2_q"], inputs["w2_scale"], inputs["w2_zero"])

    in_maps = []
    for c in range(NCORES):
        sl = slice(c * IS, (c + 1) * IS)
        w1T = w1[sl].T   # [H, IS] f32
        w3T = w3[sl].T   # [H, IS] f32
        # chunk layout [G0,U0,G1,U1,G2,U2,M]: 3 pairs of full CW-wide
        # gate/up chunks, then one mixed [256g|256u] chunk (act column
        # order is unchanged: pair i covers original act cols i*CW..)
        w13T = np.empty((H, 2 * IS), dtype=np.float32)
        NF = IS // CW  # 3 full pairs
        for i in range(NF):
            w13T[:, (2 * i) * CW:(2 * i + 1) * CW] = w1T[:, i * CW:(i + 1) * CW]
            w13T[:, (2 * i + 1) * CW:(2 * i + 2) * CW] = w3T[:, i * CW:(i + 1) * CW]
        w13T[:, 2 * NF * CW:2 * NF * CW + IC] = w1T[:, NF * CW:]
        w13T[:, 2 * NF * CW + IC:] = w3T[:, NF * CW:]
        # per-column fp8 re-quantization; scale applied to the psum output
        colmax = np.abs(w13T).max(axis=0)
        colmax[colmax == 0] = 1.0
        w13q = np.clip(w13T * (192.0 / colmax), -240, 240).astype(E4)
        csB = np.ascontiguousarray(
            np.broadcast_to(
                (colmax / (192.0 * 32.0)).astype(np.float16).reshape(NC1, CW),
                (P, NC1, CW),
            )
        )
        # [p, k, c*CW] -> [p, c, kp, 2, CW] chunk-contiguous per partition
        w13_s = np.ascontiguousarray(
            _stripe(w13q, KT)
            .reshape(P, KT, NC1, CW)
            .transpose(0, 2, 1, 3)
            .reshape(P, NC1, KP, 2, CW)
        )
        w2T = np.ascontiguousarray(w2[:, sl].T) * 16.0  # [IS, H] f32, fp8-range
        # act/16 * w2*16 -> unit output scale: evac is a plain copy
        w2_s = np.ascontiguousarray(
            _stripe(np.clip(w2T, -240, 240), IT)
            .reshape(P, IT, NH2, CW)
            .transpose(0, 2, 1, 3)                  # [P, hc, ik, CW]
            .reshape(P, NH2, IT // 2, 2, CW)        # ik -> (pair, j)
        ).astype(E4)
        in_maps.append(
            {
                "x8T": x8T,
                "e8T": e8T,
                "ident": ident_np,
                "w13": w13_s,
                "cs": csB,
                "w2": w2_s,
            }
        )
    return in_maps


_CACHE = {}


def run_on_hw(inputs, cfg=None, trace=False, trace_kwargs=None):
    from concourse.bass_utils import run_bass_kernel_spmd

    cfg = dict(FULL_CFG if cfg is None else cfg)
    key = tuple(sorted(cfg.items()))
    if key not in _CACHE:
        _CACHE[key] = build_nc(cfg)
    nc = _CACHE[key]
    in_maps = make_in_maps(inputs)
    res = run_bass_kernel_spmd(
        nc,
        in_maps,
        list(range(NCORES)),
        trace=trace,
        **(trace_kwargs or {}),
    )
    return res


def gather_out(res):
    return np.sum(
        [np.asarray(res.results[c]["out"], dtype=np.float32) for c in range(NCORES)],
        axis=0,
    )


def kernel(**inputs) -> np.ndarray:
    res = run_on_hw(inputs)
    return gather_out(res)
